# revision 6
# baseline (speedup 1.0000x reference)
"""Trainium2 Bass kernel for nn_ClassifyModelMOE (conv feature extractor +
top-3-of-5 MoE + softmax head). Data-parallel over batch across 8 cores.

Self-contained: hardcodes all shapes; builds Toeplitz-expanded conv weights on
the host; runs one SPMD Bass/Tile program on cores 0-7 via run_bass_kernel_spmd.
"""
import os
import sys

sys.path.insert(0, "/opt/trn_rl_repo")

import numpy as np
import ml_dtypes

import concourse.bacc as bacc
import concourse.mybir as mybir
import concourse.tile as tile
from concourse.bass_utils import run_bass_kernel_spmd
from concourse.masks import make_identity

F32 = mybir.dt.float32
F32R = mybir.dt.float32r
BF16 = mybir.dt.bfloat16
AF = mybir.ActivationFunctionType
ALU = mybir.AluOpType
AX = mybir.AxisListType

NCORES = 8
B = 8192
BC = B // NCORES          # tokens per core
NB = 512                  # batch chunk (columns per matmul)
NCH = BC // NB            # chunks per core
NE, TOPK = 5, 3
DH = 128

# conv1 output geometry: 16ch x 24x24; M-layout (per output row y):
#   Mc0: even x = 2j, j=0..8   -> m = j*16 + o        (128)
#   Mc1: [even j=8..12 | odd j=8..12] -> 64+64        (128)
#   Mc2: odd x = 2j+1, j=0..8  -> m = j*16 + o        (128)
# pooled row tiles: PP0 = j 0..8 (128 parts: j*16+c), PP1 = j 8..12 (64 parts)
# conv2 output (per row y): M = xout*32 + o2:
#   Mc0: xout 0..4 (128), Mc1: xout 4..8 (128),
#   Mc2pair: [y0: xout 8..10 | y1: xout 8..10] (64+64)


def _conv1_cols():
    """(x, o) per (mc, col) for the conv1 M-layout."""
    cols = {0: [], 1: [], 2: []}
    for j in range(8):
        for o in range(16):
            cols[0].append((2 * j, o))
            cols[2].append((2 * j + 1, o))
    for j in range(8, 12):
        for o in range(16):
            cols[1].append((2 * j, o))
    for j in range(8, 12):
        for o in range(16):
            cols[1].append((2 * j + 1, o))
    return cols


def _host_prep(x, conv1_w, conv1_b, conv2_w, conv2_b, gate_w, gate_b,
               e1_w, e1_b, e2_w, e2_b, sm_w, sm_b):
    x = np.asarray(x, np.float32)
    conv1_w = np.asarray(conv1_w, np.float32)
    conv2_w = np.asarray(conv2_w, np.float32)
    gate_w = np.asarray(gate_w, np.float32)
    e1_w = np.asarray(e1_w, np.float32)
    e2_w = np.asarray(e2_w, np.float32)

    # x padded: col 32*r + c, bf16
    xp = np.zeros((B, 1024), np.float32)
    xr = x.reshape(B, 28, 28)
    for r in range(28):
        xp[:, 32 * r:32 * r + 28] = xr[:, r, :]
    xp = xp.astype(ml_dtypes.bfloat16)

    cols = _conv1_cols()
    # w1c1 [128, 3*128]: K-row = 32*s + ci (s=0..4 -> dy), cols per Mc
    w1c1 = np.zeros((128, 384), np.float32)
    w1c2 = np.zeros((28, 384), np.float32)
    for mc in range(3):
        for ci_col, (xx, o) in enumerate(cols[mc]):
            col = 128 * mc + ci_col
            for s in range(4):
                for dx in range(5):
                    ci = xx + dx
                    w1c1[32 * s + ci, col] = conv1_w[o, 0, s, dx]
            for dx in range(5):
                w1c2[xx + dx, col] = conv1_w[o, 0, 4, dx]
    w1c1 = w1c1.astype(ml_dtypes.bfloat16)
    w1c2 = w1c2.astype(ml_dtypes.bfloat16)

    # conv2 toeplitz: pooled row layout p = j*16 + c (PP0: j<8), (j-8)*16+c (PP1)
    # w2p0 [3, 128, 256]: r taps, cols [Mc0 | Mc1]
    w2p0 = np.zeros((3, 128, 256), np.float32)
    w2p1mc1 = np.zeros((3, 64, 128), np.float32)
    w2p1mc2 = np.zeros((4, 64, 128), np.float32)
    for r in range(3):
        for j in range(8):
            for c in range(16):
                p = j * 16 + c
                for mci, xobase in ((0, 0), (1, 4)):
                    for xo in range(xobase, xobase + 4):
                        dx = j - xo
                        if 0 <= dx < 3:
                            for o2 in range(32):
                                w2p0[r, p, 128 * mci + (xo - xobase) * 32 + o2] = \
                                    conv2_w[o2, c, r, dx]
        for j in range(8, 12):
            for c in range(16):
                p = (j - 8) * 16 + c
                for xo in range(4, 8):
                    dx = j - xo
                    if 0 <= dx < 3:
                        for o2 in range(32):
                            w2p1mc1[r, p, (xo - 4) * 32 + o2] = conv2_w[o2, c, r, dx]
    for rr in range(4):
        for b_ in range(2):
            r = rr - b_
            if not (0 <= r < 3):
                continue
            for j in range(8, 12):
                for c in range(16):
                    p = (j - 8) * 16 + c
                    for xo in range(8, 10):
                        dx = j - xo
                        if 0 <= dx < 3:
                            for o2 in range(32):
                                w2p1mc2[rr, p, 64 * b_ + (xo - 8) * 32 + o2] = \
                                    conv2_w[o2, c, r, dx]

    # h feature permutation: our flat index (tile*128+part) -> reference f = o2*100 + y*10 + xo
    perm = np.zeros(3200, np.int64)
    for P in range(5):
        y0, y1 = 2 * P, 2 * P + 1
        tiles = []
        for yy in (y0, y1):
            for xobase in (0, 4):
                tiles.append([(yy, xo, o2) for xo in range(xobase, xobase + 4)
                              for o2 in range(32)])
        t4 = [(y0, xo, o2) for xo in range(8, 10) for o2 in range(32)] + \
             [(y1, xo, o2) for xo in range(8, 10) for o2 in range(32)]
        order = [tiles[0], tiles[1], tiles[2], tiles[3], t4]
        for ti, tl in enumerate(order):
            for p, (yy, xo, o2) in enumerate(tl):
                perm[(5 * P + ti) * 128 + p] = o2 * 100 + yy * 10 + xo
    e1wp = e1_w[:, perm, :].reshape(NE, 25, 128, DH).astype(np.float32)
    gwp = gate_w[perm, :].reshape(25, 128, NE).astype(np.float32)

    b1col = np.asarray(conv1_b, np.float32)[np.arange(128) % 16].reshape(128, 1)
    b2col = np.asarray(conv2_b, np.float32)[np.arange(128) % 32].reshape(128, 1)
    gbcol = np.asarray(gate_b, np.float32).reshape(NE, 1)
    e1bT = np.asarray(e1_b, np.float32).T.copy()      # [128, 5]
    e2bT = np.asarray(e2_b, np.float32).T.copy()      # [128, 5]
    smw = np.asarray(sm_w, np.float32)                # [128, 10]
    smb5 = np.tile(np.asarray(sm_b, np.float32), 5).reshape(1, 50)

    bf = ml_dtypes.bfloat16
    weights = dict(
        w1c1=w1c1, w1c2=w1c2,
        w2p0=np.ascontiguousarray(w2p0.transpose(1, 0, 2)).reshape(128, 768).astype(bf),
        w2p1mc1=np.ascontiguousarray(w2p1mc1.transpose(1, 0, 2)).reshape(64, 384).astype(bf),
        w2p1mc2=np.ascontiguousarray(w2p1mc2.transpose(1, 0, 2)).reshape(64, 512).astype(bf),
        e1wp=e1wp.astype(bf),
        gwp=np.ascontiguousarray(gwp.transpose(1, 0, 2)).reshape(128, 125).astype(bf),
        e2w=np.ascontiguousarray(e2_w.astype(np.float32).transpose(1, 0, 2)).reshape(128, 640).astype(bf),
        b1col=b1col, b2col=b2col,
        gbcol=gbcol, e1bT=e1bT, e2bT=e2bT, smw=smw.astype(bf), smb5=smb5.astype(bf))
    return xp, weights


def _build_nc():
    nc = bacc.Bacc("TRN2", target_bir_lowering=False, debug=False)
    d = {}
    def din(name, shape, dt):
        d[name] = nc.dram_tensor(name, list(shape), dt, kind="ExternalInput").ap()
    din("xp", (BC, 1024), BF16)
    din("w1c1", (128, 384), BF16)
    din("w1c2", (28, 384), BF16)
    din("w2p0", (128, 768), BF16)
    din("w2p1mc1", (64, 384), BF16)
    din("w2p1mc2", (64, 512), BF16)
    din("e1wp", (NE, 25, 128, DH), BF16)
    din("gwp", (128, 125), BF16)
    din("e2w", (128, 640), BF16)
    din("b1col", (128, 1), F32)
    din("b2col", (128, 1), F32)
    din("gbcol", (NE, 1), F32)
    din("e1bT", (128, NE), F32)
    din("e2bT", (128, NE), F32)
    din("smw", (128, 10), BF16)
    din("smb5", (1, 50), BF16)
    out_d = nc.dram_tensor("out", [BC, 10], F32, kind="ExternalOutput").ap()

    with tile.TileContext(nc) as tc:
        _emit(nc, tc, d, out_d)
    nc.compile()
    return nc


def _emit(nc, tc, d, out_d):
    import contextlib
    ctx = contextlib.ExitStack()
    with ctx:
        wpool = ctx.enter_context(tc.tile_pool(name="wpool", bufs=1))
        xtp = ctx.enter_context(tc.tile_pool(name="xtp", bufs=28))
        rpool = ctx.enter_context(tc.tile_pool(name="rpool", bufs=8))
        tpool = ctx.enter_context(tc.tile_pool(name="tpool", bufs=4))
        shp = ctx.enter_context(tc.tile_pool(name="shp", bufs=2))
        pp0p = ctx.enter_context(tc.tile_pool(name="pp0p", bufs=7))
        pp1p = ctx.enter_context(tc.tile_pool(name="pp1p", bufs=7))
        hpool = ctx.enter_context(tc.tile_pool(name="hpool", bufs=25))
        e1wpool = ctx.enter_context(tc.tile_pool(name="e1wpool", bufs=10))
        h1p = ctx.enter_context(tc.tile_pool(name="h1p", bufs=2))
        h2p = ctx.enter_context(tc.tile_pool(name="h2p", bufs=5))
        gp = ctx.enter_context(tc.tile_pool(name="gp", bufs=2))
        smallp = ctx.enter_context(tc.tile_pool(name="smallp", bufs=16))
        c1ps = ctx.enter_context(tc.tile_pool(name="c1ps", bufs=3, space="PSUM"))
        c2ps = ctx.enter_context(tc.tile_pool(name="c2ps", bufs=2, space="PSUM"))
        gps_p = ctx.enter_context(tc.tile_pool(name="gps_p", bufs=1, space="PSUM"))
        exps = ctx.enter_context(tc.tile_pool(name="exps", bufs=1, space="PSUM"))
        hdps = ctx.enter_context(tc.tile_pool(name="hdps", bufs=1, space="PSUM"))

        # resident weights
        w1c1 = wpool.tile([128, 384], BF16); nc.sync.dma_start(w1c1[:], d["w1c1"][:])
        w1c2 = wpool.tile([28, 384], BF16); nc.sync.dma_start(w1c2[:], d["w1c2"][:])
        w2p0 = wpool.tile([128, 3 * 256], BF16)
        nc.sync.dma_start(w2p0[:], d["w2p0"][:])
        w2p1a = wpool.tile([64, 3 * 128], BF16)
        nc.sync.dma_start(w2p1a[:], d["w2p1mc1"][:])
        w2p1b = wpool.tile([64, 4 * 128], BF16)
        nc.sync.dma_start(w2p1b[:], d["w2p1mc2"][:])
        gw = wpool.tile([128, 25 * NE], BF16)
        nc.sync.dma_start(gw[:], d["gwp"][:])
        e2w = wpool.tile([128, NE * DH], BF16)
        nc.sync.dma_start(e2w[:], d["e2w"][:])
        b1c = wpool.tile([128, 1], F32); nc.sync.dma_start(b1c[:], d["b1col"][:])
        b2c = wpool.tile([128, 1], F32); nc.sync.dma_start(b2c[:], d["b2col"][:])
        gbc = wpool.tile([NE, 1], F32); nc.sync.dma_start(gbc[:], d["gbcol"][:])
        e1bT = wpool.tile([128, NE], F32); nc.sync.dma_start(e1bT[:], d["e1bT"][:])
        e2bT = wpool.tile([128, NE], F32); nc.sync.dma_start(e2bT[:], d["e2bT"][:])
        smw = wpool.tile([128, 10], BF16); nc.sync.dma_start(smw[:], d["smw"][:])
        smb5 = wpool.tile([1, 50], BF16); nc.sync.dma_start(smb5[:], d["smb5"][:])
        ident = wpool.tile([128, 128], F32)
        make_identity(nc, ident[:])
        ones = wpool.tile([1, 128], BF16)
        nc.scalar.activation(ones[:], e2w[0:1, 0:128], AF.Copy, scale=0.0, bias=1.0)

        for ch in range(NCH):
            b0 = ch * NB
            # ---- x^T tiles (DMA transpose) ----
            xT = []
            for k in range(28):
                t = xtp.tile([128, NB], BF16, tag="xT")
                nc.sync.dma_start(t[:], d["xp"][b0:b0 + NB, 32 * k:32 * k + 128],
                                  transpose=True)
                xT.append(t)

            # ---- conv1 + relu + pool ----
            pp0, pp1 = [], []
            for Y in range(12):
                rt = {}
                for yy in (2 * Y, 2 * Y + 1):
                    for mc in range(3):
                        ps = c1ps.tile([128, NB], F32, tag="ps")
                        nc.tensor.matmul(ps[:], w1c1[:, 128 * mc:128 * mc + 128],
                                         xT[yy][:], start=True, stop=False)
                        nc.tensor.matmul(ps[:], w1c2[:, 128 * mc:128 * mc + 128],
                                         xT[yy + 4][0:28, :], start=False, stop=True)
                        r = rpool.tile([128, NB], BF16, tag="r")
                        nc.scalar.activation(r[:], ps[:], AF.Relu, bias=b1c[:, 0:1])
                        rt[(yy, mc)] = r
                tm = []
                for mc in range(3):
                    t = tpool.tile([128, NB], BF16, tag="tm")
                    nc.vector.tensor_tensor(t[:], rt[(2 * Y, mc)][:],
                                            rt[(2 * Y + 1, mc)][:], op=ALU.max)
                    tm.append(t)
                p0 = pp0p.tile([128, NB], BF16, tag="pp0")
                nc.vector.tensor_tensor(p0[:], tm[0][:], tm[2][:], op=ALU.max)
                sh = shp.tile([64, NB], BF16, tag="sh")
                nc.sync.dma_start(sh[:], tm[1][64:128, :])
                p1 = pp1p.tile([64, NB], BF16, tag="pp1")
                nc.vector.tensor_tensor(p1[:], tm[1][0:64, :], sh[:], op=ALU.max)
                pp0.append(p0)
                pp1.append(p1)

            # ---- conv2 + relu -> h tiles ----
            htiles = []
            for P in range(5):
                y0 = 2 * P
                for yy in (y0, y0 + 1):
                    for mci in range(2):
                        ps = c2ps.tile([128, NB], F32, tag="ps")
                        for r in range(3):
                            nc.tensor.matmul(
                                ps[:], w2p0[:, 256 * r + 128 * mci:256 * r + 128 * mci + 128],
                                pp0[yy + r][:], start=(r == 0),
                                stop=(mci == 0 and r == 2))
                        if mci == 1:
                            for r in range(3):
                                nc.tensor.matmul(ps[:], w2p1a[:, 128 * r:128 * r + 128],
                                                 pp1[yy + r][:], start=False,
                                                 stop=(r == 2))
                        h = hpool.tile([128, NB], BF16, tag="h")
                        nc.scalar.activation(h[:], ps[:], AF.Relu, bias=b2c[:, 0:1])
                        htiles.append(h)
                ps = c2ps.tile([128, NB], F32, tag="ps")
                for rr in range(4):
                    nc.tensor.matmul(ps[:], w2p1b[:, 128 * rr:128 * rr + 128],
                                     pp1[y0 + rr][:], start=(rr == 0), stop=(rr == 3))
                h = hpool.tile([128, NB], BF16, tag="h")
                nc.scalar.activation(h[:], ps[:], AF.Relu, bias=b2c[:, 0:1])
                htiles.append(h)
                # reorder: we appended [Mc0_y0, Mc1_y0, Mc0_y1, Mc1_y1] then Mc2pair
            # htiles order now matches perm construction ✓

            # ---- gate ----
            gps = gps_p.tile([NE, NB], F32, tag="gps")
            for kc in range(25):
                nc.tensor.matmul(gps[:], gw[:, NE * kc:NE * kc + NE], htiles[kc][:],
                                 start=(kc == 0), stop=(kc == 24))
            gsb = gp.tile([NE, NB], F32, tag="gsb")
            nc.scalar.activation(gsb[:], gps[:], AF.Identity, bias=gbc[:, 0:1])

            # ---- experts ----
            h2t = []
            for e in range(NE):
                h1ps = exps.tile([128, NB], F32, tag="exps")
                for kc in range(25):
                    wt = e1wpool.tile([128, DH], BF16, tag="e1w")
                    nc.sync.dma_start(wt[:], d["e1wp"][e, kc])
                    nc.tensor.matmul(h1ps[:], wt[:], htiles[kc][:],
                                     start=(kc == 0), stop=(kc == 24))
                h1 = h1p.tile([128, NB], BF16, tag="h1")
                nc.scalar.activation(h1[:], h1ps[:], AF.Tanh, bias=e1bT[:, e:e + 1])
                h2ps = exps.tile([128, NB], F32, tag="exps")
                nc.tensor.matmul(h2ps[:], e2w[:, DH * e:DH * e + DH], h1[:],
                                 start=True, stop=True)
                h2 = h2p.tile([128, NB], BF16, tag="h2")
                nc.scalar.activation(h2[:], h2ps[:], AF.Tanh, bias=e2bT[:, e:e + 1])
                h2t.append(h2)

            # ---- per-token-chunk: gating weights, head, softmax ----
            for t4 in range(NB // 128):
                tok = slice(128 * t4, 128 * t4 + 128)
                gtp = hdps.tile([128, NE], F32, tag="hd")
                nc.tensor.transpose(gtp[:], gsb[:, tok], ident[0:NE, 0:NE])
                s = smallp.tile([128, NE], F32, tag="s")
                nc.scalar.activation(s[:], gtp[:], AF.Copy)
                mx = smallp.tile([128, 1], F32, tag="mx")
                nc.vector.reduce_max(mx[:], s[:], axis=AX.X)
                nmx = smallp.tile([128, 1], F32, tag="nmx")
                nc.vector.tensor_scalar_mul(nmx[:], mx[:], -1.0)
                ex = smallp.tile([128, NE], F32, tag="ex")
                nc.scalar.activation(ex[:], s[:], AF.Exp, bias=nmx[:, 0:1])
                gt = smallp.tile([128, NE * NE], F32, tag="gt")
                a_b = ex[:].unsqueeze(1).broadcast_to([128, NE, NE])
                b_b = ex[:].unsqueeze(2).broadcast_to([128, NE, NE])
                nc.vector.tensor_tensor(gt[:].rearrange("p (i j) -> p i j", i=NE),
                                        a_b, b_b, op=ALU.is_gt)
                rank = smallp.tile([128, NE], F32, tag="rank")
                nc.vector.reduce_sum(rank[:], gt[:].rearrange("p (i j) -> p i j", i=NE),
                                     axis=AX.X)
                m01 = smallp.tile([128, NE], F32, tag="m01")
                nc.vector.tensor_scalar(m01[:], rank[:], float(TOPK) - 0.5, None,
                                        op0=ALU.is_le)
                wun = smallp.tile([128, NE], F32, tag="wun")
                nc.vector.tensor_mul(wun[:], ex[:], m01[:])
                ssum = smallp.tile([128, 1], F32, tag="ssum")
                nc.vector.reduce_sum(ssum[:], wun[:], axis=AX.X)
                rinv = smallp.tile([128, 1], F32, tag="rinv")
                nc.vector.reciprocal(rinv[:], ssum[:])
                wfin = smallp.tile([128, NE], F32, tag="wfin")
                nc.vector.tensor_scalar_mul(wfin[:], wun[:], rinv[:, 0:1])

                lep = hdps.tile([128, 50], F32, tag="hd")
                nc.tensor.matmul(lep[:, 0:50], ones[:], smb5[:], start=True, stop=False)
                for e in range(NE):
                    nc.tensor.matmul(lep[:, 10 * e:10 * e + 10], h2t[e][:, tok],
                                     smw[:], start=False, stop=(e == NE - 1))
                scl = smallp.tile([128, 50], F32, tag="scl")
                for e in range(NE):
                    nc.vector.tensor_scalar_mul(scl[:, 10 * e:10 * e + 10],
                                                lep[:, 10 * e:10 * e + 10],
                                                wfin[:, e:e + 1])
                logit = smallp.tile([128, 10], F32, tag="logit")
                nc.vector.reduce_sum(logit[:], scl[:].rearrange("p (e k) -> p k e", e=NE),
                                     axis=AX.X)
                lmx = smallp.tile([128, 1], F32, tag="lmx")
                nc.vector.reduce_max(lmx[:], logit[:], axis=AX.X)
                nlmx = smallp.tile([128, 1], F32, tag="nlmx")
                nc.vector.tensor_scalar_mul(nlmx[:], lmx[:], -1.0)
                lex = smallp.tile([128, 10], F32, tag="lex")
                nc.scalar.activation(lex[:], logit[:], AF.Exp, bias=nlmx[:, 0:1])
                lsum = smallp.tile([128, 1], F32, tag="lsum")
                nc.vector.reduce_sum(lsum[:], lex[:], axis=AX.X)
                lrinv = smallp.tile([128, 1], F32, tag="lrinv")
                nc.vector.reciprocal(lrinv[:], lsum[:])
                prob = smallp.tile([128, 10], F32, tag="prob")
                nc.vector.tensor_scalar_mul(prob[:], lex[:], lrinv[:, 0:1])
                nc.sync.dma_start(out_d[b0 + 128 * t4:b0 + 128 * t4 + 128, :], prob[:])


_NC_CACHE = None


def _get_nc():
    global _NC_CACHE
    if _NC_CACHE is None:
        _NC_CACHE = _build_nc()
    return _NC_CACHE


def kernel(**inputs):
    xp, w = _host_prep(**inputs)
    in_maps = []
    for c in range(NCORES):
        m = {"xp": np.ascontiguousarray(xp[c * BC:(c + 1) * BC])}
        m.update(w)
        in_maps.append(m)
    nc = _get_nc()
    trace = bool(int(os.environ.get("KERNEL_TRACE", "0")))
    res = run_bass_kernel_spmd(nc, in_maps, list(range(NCORES)), trace=trace)
    kernel.last_results = res
    out = np.concatenate([res.results[c]["out"] for c in range(NCORES)], axis=0)
    return out.astype(np.float32)


# revision 7
# speedup vs baseline: 72.4792x; 72.4792x over previous
"""Trainium2 Bass kernel for nn_ClassifyModelMOE (conv feature extractor +
top-3-of-5 MoE + softmax head). Data-parallel over batch across 8 cores.

Self-contained: hardcodes all shapes; builds Toeplitz-expanded conv weights on
the host; runs one SPMD Bass/Tile program on cores 0-7 via run_bass_kernel_spmd.
"""
import os
import sys

sys.path.insert(0, "/opt/trn_rl_repo")

import numpy as np
import ml_dtypes

import concourse.bacc as bacc
import concourse.mybir as mybir
import concourse.tile as tile
from concourse.bass_utils import run_bass_kernel_spmd
from concourse.masks import make_identity

F32 = mybir.dt.float32
F32R = mybir.dt.float32r
BF16 = mybir.dt.bfloat16
AF = mybir.ActivationFunctionType
ALU = mybir.AluOpType
AX = mybir.AxisListType

NCORES = 8
B = 8192
BC = B // NCORES          # tokens per core
NB = 512                  # batch chunk (columns per matmul)
NCH = BC // NB            # chunks per core
NE, TOPK = 5, 3
DH = 128

# conv1 output geometry: 16ch x 24x24; M-layout (per output row y):
#   Mc0: even x = 2j, j=0..8   -> m = j*16 + o        (128)
#   Mc1: [even j=8..12 | odd j=8..12] -> 64+64        (128)
#   Mc2: odd x = 2j+1, j=0..8  -> m = j*16 + o        (128)
# pooled row tiles: PP0 = j 0..8 (128 parts: j*16+c), PP1 = j 8..12 (64 parts)
# conv2 output (per row y): M = xout*32 + o2:
#   Mc0: xout 0..4 (128), Mc1: xout 4..8 (128),
#   Mc2pair: [y0: xout 8..10 | y1: xout 8..10] (64+64)


def _conv1_cols():
    """(x, o) per (mc, col) for the conv1 M-layout."""
    cols = {0: [], 1: [], 2: []}
    for j in range(8):
        for o in range(16):
            cols[0].append((2 * j, o))
            cols[2].append((2 * j + 1, o))
    for j in range(8, 12):
        for o in range(16):
            cols[1].append((2 * j, o))
    for j in range(8, 12):
        for o in range(16):
            cols[1].append((2 * j + 1, o))
    return cols


def _host_prep(x, conv1_w, conv1_b, conv2_w, conv2_b, gate_w, gate_b,
               e1_w, e1_b, e2_w, e2_b, sm_w, sm_b):
    x = np.asarray(x, np.float32)
    conv1_w = np.asarray(conv1_w, np.float32)
    conv2_w = np.asarray(conv2_w, np.float32)
    gate_w = np.asarray(gate_w, np.float32)
    e1_w = np.asarray(e1_w, np.float32)
    e2_w = np.asarray(e2_w, np.float32)

    # x padded: col 32*r + c, bf16
    xp = np.zeros((B, 1024), np.float32)
    xr = x.reshape(B, 28, 28)
    for r in range(28):
        xp[:, 32 * r:32 * r + 28] = xr[:, r, :]
    xp = xp.astype(ml_dtypes.bfloat16)

    cols = _conv1_cols()
    # w1c1 [128, 3*128]: K-row = 32*s + ci (s=0..4 -> dy), cols per Mc
    w1c1 = np.zeros((128, 384), np.float32)
    w1c2 = np.zeros((28, 384), np.float32)
    for mc in range(3):
        for ci_col, (xx, o) in enumerate(cols[mc]):
            col = 128 * mc + ci_col
            for s in range(4):
                for dx in range(5):
                    ci = xx + dx
                    w1c1[32 * s + ci, col] = conv1_w[o, 0, s, dx]
            for dx in range(5):
                w1c2[xx + dx, col] = conv1_w[o, 0, 4, dx]
    w1c1 = w1c1.astype(ml_dtypes.bfloat16)
    w1c2 = w1c2.astype(ml_dtypes.bfloat16)

    # conv2 toeplitz: pooled row layout p = j*16 + c (PP0: j<8), (j-8)*16+c (PP1)
    # w2p0 [3, 128, 256]: r taps, cols [Mc0 | Mc1]
    w2p0 = np.zeros((3, 128, 256), np.float32)
    w2p1mc1 = np.zeros((3, 64, 128), np.float32)
    w2p1mc2 = np.zeros((4, 64, 128), np.float32)
    for r in range(3):
        for j in range(8):
            for c in range(16):
                p = j * 16 + c
                for mci, xobase in ((0, 0), (1, 4)):
                    for xo in range(xobase, xobase + 4):
                        dx = j - xo
                        if 0 <= dx < 3:
                            for o2 in range(32):
                                w2p0[r, p, 128 * mci + (xo - xobase) * 32 + o2] = \
                                    conv2_w[o2, c, r, dx]
        for j in range(8, 12):
            for c in range(16):
                p = (j - 8) * 16 + c
                for xo in range(4, 8):
                    dx = j - xo
                    if 0 <= dx < 3:
                        for o2 in range(32):
                            w2p1mc1[r, p, (xo - 4) * 32 + o2] = conv2_w[o2, c, r, dx]
    for rr in range(4):
        for b_ in range(2):
            r = rr - b_
            if not (0 <= r < 3):
                continue
            for j in range(8, 12):
                for c in range(16):
                    p = (j - 8) * 16 + c
                    for xo in range(8, 10):
                        dx = j - xo
                        if 0 <= dx < 3:
                            for o2 in range(32):
                                w2p1mc2[rr, p, 64 * b_ + (xo - 8) * 32 + o2] = \
                                    conv2_w[o2, c, r, dx]

    # h feature permutation: our flat index (tile*128+part) -> reference f = o2*100 + y*10 + xo
    perm = np.zeros(3200, np.int64)
    for P in range(5):
        y0, y1 = 2 * P, 2 * P + 1
        tiles = []
        for yy in (y0, y1):
            for xobase in (0, 4):
                tiles.append([(yy, xo, o2) for xo in range(xobase, xobase + 4)
                              for o2 in range(32)])
        t4 = [(y0, xo, o2) for xo in range(8, 10) for o2 in range(32)] + \
             [(y1, xo, o2) for xo in range(8, 10) for o2 in range(32)]
        order = [tiles[0], tiles[1], tiles[2], tiles[3], t4]
        for ti, tl in enumerate(order):
            for p, (yy, xo, o2) in enumerate(tl):
                perm[(5 * P + ti) * 128 + p] = o2 * 100 + yy * 10 + xo
    e1wp = e1_w[:, perm, :].reshape(NE, 25, 128, DH).astype(np.float32)
    gwp = gate_w[perm, :].reshape(25, 128, NE).astype(np.float32)

    b1col = np.asarray(conv1_b, np.float32)[np.arange(128) % 16].reshape(128, 1)
    b2col = np.asarray(conv2_b, np.float32)[np.arange(128) % 32].reshape(128, 1)
    gbcol = np.asarray(gate_b, np.float32).reshape(NE, 1)
    e1bT = np.asarray(e1_b, np.float32).T.copy()      # [128, 5]
    e2bT = np.asarray(e2_b, np.float32).T.copy()      # [128, 5]
    smw = np.asarray(sm_w, np.float32)                # [128, 10]
    smb5 = np.tile(np.asarray(sm_b, np.float32), 5).reshape(1, 50)

    bf = ml_dtypes.bfloat16
    weights = dict(
        w1c1=w1c1, w1c2=w1c2,
        w2p0=np.ascontiguousarray(w2p0.transpose(1, 0, 2)).reshape(128, 768).astype(bf),
        w2p1mc1=np.ascontiguousarray(w2p1mc1.transpose(1, 0, 2)).reshape(64, 384).astype(bf),
        w2p1mc2=np.ascontiguousarray(w2p1mc2.transpose(1, 0, 2)).reshape(64, 512).astype(bf),
        e1wp=e1wp.astype(bf),
        gwp=np.ascontiguousarray(gwp.transpose(1, 0, 2)).reshape(128, 125).astype(bf),
        e2w=np.ascontiguousarray(e2_w.astype(np.float32).transpose(1, 0, 2)).reshape(128, 640).astype(bf),
        b1col=b1col, b2col=b2col,
        gbcol=gbcol, e1bT=e1bT, e2bT=e2bT, smw=smw.astype(bf), smb5=smb5.astype(bf))
    return xp, weights


def _build_nc(loop_reps=None):
    nc = bacc.Bacc("TRN2", target_bir_lowering=False, debug=False)
    d = {}
    def din(name, shape, dt):
        d[name] = nc.dram_tensor(name, list(shape), dt, kind="ExternalInput").ap()
    din("xp", (BC, 1024), BF16)
    din("w1c1", (128, 384), BF16)
    din("w1c2", (28, 384), BF16)
    din("w2p0", (128, 768), BF16)
    din("w2p1mc1", (64, 384), BF16)
    din("w2p1mc2", (64, 512), BF16)
    din("e1wp", (NE, 25, 128, DH), BF16)
    din("gwp", (128, 125), BF16)
    din("e2w", (128, 640), BF16)
    din("b1col", (128, 1), F32)
    din("b2col", (128, 1), F32)
    din("gbcol", (NE, 1), F32)
    din("e1bT", (128, NE), F32)
    din("e2bT", (128, NE), F32)
    din("smw", (128, 10), BF16)
    din("smb5", (1, 50), BF16)
    out_d = nc.dram_tensor("out", [BC, 10], F32, kind="ExternalOutput").ap()

    with tile.TileContext(nc) as tc:
        _emit(nc, tc, d, out_d, loop_reps=loop_reps)
    nc.compile()
    return nc


def _emit(nc, tc, d, out_d, loop_reps=None):
    import contextlib
    ctx = contextlib.ExitStack()
    with ctx:
        wpool = ctx.enter_context(tc.tile_pool(name="wpool", bufs=1))
        xtp = ctx.enter_context(tc.tile_pool(name="xtp", bufs=28))
        rpool = ctx.enter_context(tc.tile_pool(name="rpool", bufs=8))
        tpool = ctx.enter_context(tc.tile_pool(name="tpool", bufs=4))
        shp = ctx.enter_context(tc.tile_pool(name="shp", bufs=2))
        pp0p = ctx.enter_context(tc.tile_pool(name="pp0p", bufs=7))
        pp1p = ctx.enter_context(tc.tile_pool(name="pp1p", bufs=7))
        hpool = ctx.enter_context(tc.tile_pool(name="hpool", bufs=25))
        e1wpool = ctx.enter_context(tc.tile_pool(name="e1wpool", bufs=10))
        h1p = ctx.enter_context(tc.tile_pool(name="h1p", bufs=2))
        h2p = ctx.enter_context(tc.tile_pool(name="h2p", bufs=5))
        gp = ctx.enter_context(tc.tile_pool(name="gp", bufs=2))
        smallp = ctx.enter_context(tc.tile_pool(name="smallp", bufs=16))
        c1ps = ctx.enter_context(tc.tile_pool(name="c1ps", bufs=3, space="PSUM"))
        c2ps = ctx.enter_context(tc.tile_pool(name="c2ps", bufs=2, space="PSUM"))
        gps_p = ctx.enter_context(tc.tile_pool(name="gps_p", bufs=1, space="PSUM"))
        exps = ctx.enter_context(tc.tile_pool(name="exps", bufs=1, space="PSUM"))
        hdps = ctx.enter_context(tc.tile_pool(name="hdps", bufs=1, space="PSUM"))

        # resident weights
        w1c1 = wpool.tile([128, 384], BF16); nc.sync.dma_start(w1c1[:], d["w1c1"][:])
        w1c2 = wpool.tile([28, 384], BF16); nc.sync.dma_start(w1c2[:], d["w1c2"][:])
        w2p0 = wpool.tile([128, 3 * 256], BF16)
        nc.sync.dma_start(w2p0[:], d["w2p0"][:])
        w2p1a = wpool.tile([64, 3 * 128], BF16)
        nc.sync.dma_start(w2p1a[:], d["w2p1mc1"][:])
        w2p1b = wpool.tile([64, 4 * 128], BF16)
        nc.sync.dma_start(w2p1b[:], d["w2p1mc2"][:])
        gw = wpool.tile([128, 25 * NE], BF16)
        nc.sync.dma_start(gw[:], d["gwp"][:])
        e2w = wpool.tile([128, NE * DH], BF16)
        nc.sync.dma_start(e2w[:], d["e2w"][:])
        b1c = wpool.tile([128, 1], F32); nc.sync.dma_start(b1c[:], d["b1col"][:])
        b2c = wpool.tile([128, 1], F32); nc.sync.dma_start(b2c[:], d["b2col"][:])
        gbc = wpool.tile([NE, 1], F32); nc.sync.dma_start(gbc[:], d["gbcol"][:])
        e1bT = wpool.tile([128, NE], F32); nc.sync.dma_start(e1bT[:], d["e1bT"][:])
        e2bT = wpool.tile([128, NE], F32); nc.sync.dma_start(e2bT[:], d["e2bT"][:])
        smw = wpool.tile([128, 10], BF16); nc.sync.dma_start(smw[:], d["smw"][:])
        smb5 = wpool.tile([1, 50], BF16); nc.sync.dma_start(smb5[:], d["smb5"][:])
        ident = wpool.tile([128, 128], F32)
        make_identity(nc, ident[:])
        ones = wpool.tile([1, 128], BF16)
        nc.scalar.activation(ones[:], e2w[0:1, 0:128], AF.Copy, scale=0.0, bias=1.0)

        import contextlib as _ctl
        loop_cm = tc.For_i(0, loop_reps, 1) if loop_reps else _ctl.nullcontext()
        with loop_cm:
         for ch in range(NCH):
            b0 = ch * NB
            # ---- x^T tiles (DMA transpose) ----
            xT = []
            for k in range(28):
                t = xtp.tile([128, NB], BF16, tag="xT")
                nc.sync.dma_start(t[:], d["xp"][b0:b0 + NB, 32 * k:32 * k + 128],
                                  transpose=True)
                xT.append(t)

            # ---- conv1 + relu + pool ----
            pp0, pp1 = [], []
            for Y in range(12):
                rt = {}
                for yy in (2 * Y, 2 * Y + 1):
                    for mc in range(3):
                        ps = c1ps.tile([128, NB], F32, tag="ps")
                        nc.tensor.matmul(ps[:], w1c1[:, 128 * mc:128 * mc + 128],
                                         xT[yy][:], start=True, stop=False)
                        nc.tensor.matmul(ps[:], w1c2[:, 128 * mc:128 * mc + 128],
                                         xT[yy + 4][0:28, :], start=False, stop=True)
                        r = rpool.tile([128, NB], BF16, tag="r")
                        nc.scalar.activation(r[:], ps[:], AF.Relu, bias=b1c[:, 0:1])
                        rt[(yy, mc)] = r
                tm = []
                for mc in range(3):
                    t = tpool.tile([128, NB], BF16, tag="tm")
                    nc.vector.tensor_tensor(t[:], rt[(2 * Y, mc)][:],
                                            rt[(2 * Y + 1, mc)][:], op=ALU.max)
                    tm.append(t)
                p0 = pp0p.tile([128, NB], BF16, tag="pp0")
                nc.vector.tensor_tensor(p0[:], tm[0][:], tm[2][:], op=ALU.max)
                sh = shp.tile([64, NB], BF16, tag="sh")
                nc.sync.dma_start(sh[:], tm[1][64:128, :])
                p1 = pp1p.tile([64, NB], BF16, tag="pp1")
                nc.vector.tensor_tensor(p1[:], tm[1][0:64, :], sh[:], op=ALU.max)
                pp0.append(p0)
                pp1.append(p1)

            # ---- conv2 + relu -> h tiles ----
            htiles = []
            for P in range(5):
                y0 = 2 * P
                for yy in (y0, y0 + 1):
                    for mci in range(2):
                        ps = c2ps.tile([128, NB], F32, tag="ps")
                        for r in range(3):
                            nc.tensor.matmul(
                                ps[:], w2p0[:, 256 * r + 128 * mci:256 * r + 128 * mci + 128],
                                pp0[yy + r][:], start=(r == 0),
                                stop=(mci == 0 and r == 2))
                        if mci == 1:
                            for r in range(3):
                                nc.tensor.matmul(ps[:], w2p1a[:, 128 * r:128 * r + 128],
                                                 pp1[yy + r][:], start=False,
                                                 stop=(r == 2))
                        h = hpool.tile([128, NB], BF16, tag="h")
                        nc.scalar.activation(h[:], ps[:], AF.Relu, bias=b2c[:, 0:1])
                        htiles.append(h)
                ps = c2ps.tile([128, NB], F32, tag="ps")
                for rr in range(4):
                    nc.tensor.matmul(ps[:], w2p1b[:, 128 * rr:128 * rr + 128],
                                     pp1[y0 + rr][:], start=(rr == 0), stop=(rr == 3))
                h = hpool.tile([128, NB], BF16, tag="h")
                nc.scalar.activation(h[:], ps[:], AF.Relu, bias=b2c[:, 0:1])
                htiles.append(h)
                # reorder: we appended [Mc0_y0, Mc1_y0, Mc0_y1, Mc1_y1] then Mc2pair
            # htiles order now matches perm construction ✓

            # ---- gate ----
            gps = gps_p.tile([NE, NB], F32, tag="gps")
            for kc in range(25):
                nc.tensor.matmul(gps[:], gw[:, NE * kc:NE * kc + NE], htiles[kc][:],
                                 start=(kc == 0), stop=(kc == 24))
            gsb = gp.tile([NE, NB], F32, tag="gsb")
            nc.scalar.activation(gsb[:], gps[:], AF.Identity, bias=gbc[:, 0:1])

            # ---- experts ----
            h2t = []
            for e in range(NE):
                h1ps = exps.tile([128, NB], F32, tag="exps")
                for kc in range(25):
                    wt = e1wpool.tile([128, DH], BF16, tag="e1w")
                    nc.sync.dma_start(wt[:], d["e1wp"][e, kc])
                    nc.tensor.matmul(h1ps[:], wt[:], htiles[kc][:],
                                     start=(kc == 0), stop=(kc == 24))
                h1 = h1p.tile([128, NB], BF16, tag="h1")
                nc.scalar.activation(h1[:], h1ps[:], AF.Tanh, bias=e1bT[:, e:e + 1])
                h2ps = exps.tile([128, NB], F32, tag="exps")
                nc.tensor.matmul(h2ps[:], e2w[:, DH * e:DH * e + DH], h1[:],
                                 start=True, stop=True)
                h2 = h2p.tile([128, NB], BF16, tag="h2")
                nc.scalar.activation(h2[:], h2ps[:], AF.Tanh, bias=e2bT[:, e:e + 1])
                h2t.append(h2)

            # ---- per-token-chunk: gating weights, head, softmax ----
            for t4 in range(NB // 128):
                tok = slice(128 * t4, 128 * t4 + 128)
                gtp = hdps.tile([128, NE], F32, tag="hd")
                nc.tensor.transpose(gtp[:], gsb[:, tok], ident[0:NE, 0:NE])
                s = smallp.tile([128, NE], F32, tag="s")
                nc.scalar.activation(s[:], gtp[:], AF.Copy)
                mx = smallp.tile([128, 1], F32, tag="mx")
                nc.vector.reduce_max(mx[:], s[:], axis=AX.X)
                nmx = smallp.tile([128, 1], F32, tag="nmx")
                nc.vector.tensor_scalar_mul(nmx[:], mx[:], -1.0)
                ex = smallp.tile([128, NE], F32, tag="ex")
                nc.scalar.activation(ex[:], s[:], AF.Exp, bias=nmx[:, 0:1])
                gt = smallp.tile([128, NE * NE], F32, tag="gt")
                a_b = ex[:].unsqueeze(1).broadcast_to([128, NE, NE])
                b_b = ex[:].unsqueeze(2).broadcast_to([128, NE, NE])
                nc.vector.tensor_tensor(gt[:].rearrange("p (i j) -> p i j", i=NE),
                                        a_b, b_b, op=ALU.is_gt)
                rank = smallp.tile([128, NE], F32, tag="rank")
                nc.vector.reduce_sum(rank[:], gt[:].rearrange("p (i j) -> p i j", i=NE),
                                     axis=AX.X)
                m01 = smallp.tile([128, NE], F32, tag="m01")
                nc.vector.tensor_scalar(m01[:], rank[:], float(TOPK) - 0.5, None,
                                        op0=ALU.is_le)
                wun = smallp.tile([128, NE], F32, tag="wun")
                nc.vector.tensor_mul(wun[:], ex[:], m01[:])
                ssum = smallp.tile([128, 1], F32, tag="ssum")
                nc.vector.reduce_sum(ssum[:], wun[:], axis=AX.X)
                rinv = smallp.tile([128, 1], F32, tag="rinv")
                nc.vector.reciprocal(rinv[:], ssum[:])
                wfin = smallp.tile([128, NE], F32, tag="wfin")
                nc.vector.tensor_scalar_mul(wfin[:], wun[:], rinv[:, 0:1])

                lep = hdps.tile([128, 50], F32, tag="hd")
                nc.tensor.matmul(lep[:, 0:50], ones[:], smb5[:], start=True, stop=False)
                for e in range(NE):
                    nc.tensor.matmul(lep[:, 10 * e:10 * e + 10], h2t[e][:, tok],
                                     smw[:], start=False, stop=(e == NE - 1))
                scl = smallp.tile([128, 50], F32, tag="scl")
                for e in range(NE):
                    nc.vector.tensor_scalar_mul(scl[:, 10 * e:10 * e + 10],
                                                lep[:, 10 * e:10 * e + 10],
                                                wfin[:, e:e + 1])
                logit = smallp.tile([128, 10], F32, tag="logit")
                nc.vector.reduce_sum(logit[:], scl[:].rearrange("p (e k) -> p k e", e=NE),
                                     axis=AX.X)
                lmx = smallp.tile([128, 1], F32, tag="lmx")
                nc.vector.reduce_max(lmx[:], logit[:], axis=AX.X)
                nlmx = smallp.tile([128, 1], F32, tag="nlmx")
                nc.vector.tensor_scalar_mul(nlmx[:], lmx[:], -1.0)
                lex = smallp.tile([128, 10], F32, tag="lex")
                nc.scalar.activation(lex[:], logit[:], AF.Exp, bias=nlmx[:, 0:1])
                lsum = smallp.tile([128, 1], F32, tag="lsum")
                nc.vector.reduce_sum(lsum[:], lex[:], axis=AX.X)
                lrinv = smallp.tile([128, 1], F32, tag="lrinv")
                nc.vector.reciprocal(lrinv[:], lsum[:])
                prob = smallp.tile([128, 10], F32, tag="prob")
                nc.vector.tensor_scalar_mul(prob[:], lex[:], lrinv[:, 0:1])
                nc.sync.dma_start(out_d[b0 + 128 * t4:b0 + 128 * t4 + 128, :], prob[:])


_NC_CACHE = None


def _get_nc():
    global _NC_CACHE
    if _NC_CACHE is None:
        _NC_CACHE = _build_nc()
    return _NC_CACHE


def kernel(**inputs):
    xp, w = _host_prep(**inputs)
    in_maps = []
    for c in range(NCORES):
        m = {"xp": np.ascontiguousarray(xp[c * BC:(c + 1) * BC])}
        m.update(w)
        in_maps.append(m)
    nc = _get_nc()
    trace = bool(int(os.environ.get("KERNEL_TRACE", "0")))
    res = run_bass_kernel_spmd(nc, in_maps, list(range(NCORES)), trace=trace)
    kernel.last_results = res
    out = np.concatenate([res.results[c]["out"] for c in range(NCORES)], axis=0)
    return out.astype(np.float32)
